# revision 44
# baseline (speedup 1.0000x reference)
"""Trainium2 Bass kernel for a dense transformer block (pre-LN, 16-head causal
attention + 3x FFN), distributed over 8 NeuronCores.

Sharding: tensor-parallel over heads (2 heads/core, both batch elements on
every core) for LN1/QKV/attention; one 8-core AllToAll redistributes the
per-head attention context to token-parallel shards (512 tokens/core) for the
output projection, LN2 and the FFN.  Matmuls run in bf16 with f32 PSUM
accumulation; the residual stream stays f32.

All layouts are transposed ([channel, token]) on chip so every matmul
contracts over the partition dim.  LayerNorm 1 is folded into the QKV weights:
q = inv_std[t] * (x @ Wq_eff - mu[t] * colsum(Wq_eff)) + be1 @ Wq, implemented
with a rank-2 correction matmul appended to each accumulation group.
"""

import numpy as np
import ml_dtypes

B, T, C = 2, 2048, 1024
NH, H = 16, 64
FF = 3 * C
EPS = 1e-6
N_CORES = 8
TT = B * T            # 4096 tokens processed per core (head-parallel phase)
TS = TT // N_CORES    # 512 tokens per core (token-parallel phase)
HPC = NH // N_CORES   # 2 heads per core
HD2 = HPC * H         # 128

BF16 = ml_dtypes.bfloat16

_BUILT = {}

NT = TT // 128        # 32 token tiles
NKC = C // 128        # 8 channel k-tiles
NMF = FF // 128       # 24 ff tiles


def _build():
    import concourse.bacc as bacc
    import concourse.mybir as mybir
    import concourse.tile as tile
    dt = mybir.dt
    alu = mybir.AluOpType
    act = mybir.ActivationFunctionType

    nc = bacc.Bacc("TRN2", target_bir_lowering=False, debug=False,
                   num_devices=N_CORES)

    # ----- kernel I/O (per-core shards) -----
    p_x = nc.declare_dram_parameter("p_x", [TT, C], dt.bfloat16, isOutput=False)
    p_xT = nc.declare_dram_parameter("p_xT", [C, TT], dt.bfloat16, isOutput=False)
    p_xs = nc.declare_dram_parameter("p_xs", [TS, C], dt.float32, isOutput=False)
    p_wq = nc.declare_dram_parameter("p_wq", [C, HD2], dt.bfloat16, isOutput=False)
    p_wk = nc.declare_dram_parameter("p_wk", [C, HD2], dt.bfloat16, isOutput=False)
    p_wv = nc.declare_dram_parameter("p_wv", [C, HD2], dt.bfloat16, isOutput=False)
    p_cq = nc.declare_dram_parameter("p_cq", [2, HD2], dt.bfloat16, isOutput=False)
    p_ck = nc.declare_dram_parameter("p_ck", [2, HD2], dt.bfloat16, isOutput=False)
    p_cv = nc.declare_dram_parameter("p_cv", [2, HD2], dt.bfloat16, isOutput=False)
    p_womov = nc.declare_dram_parameter("p_womov", [128, NKC * C], dt.bfloat16, isOutput=False)
    p_w1blk = nc.declare_dram_parameter("p_w1blk", [NMF, C, 128], dt.bfloat16, isOutput=False)
    p_b1c = nc.declare_dram_parameter("p_b1c", [128, NMF], dt.float32, isOutput=False)
    p_w2mov = nc.declare_dram_parameter("p_w2mov", [128, NMF * C], dt.bfloat16, isOutput=False)
    p_b2 = nc.declare_dram_parameter("p_b2", [1, C], dt.bfloat16, isOutput=False)
    p_maskw = nc.declare_dram_parameter("p_maskw", [128, 4 * 512], dt.bfloat16, isOutput=False)
    p_ind2 = nc.declare_dram_parameter("p_ind2", [2, 128], dt.bfloat16, isOutput=False)
    p_ident = nc.declare_dram_parameter("p_ident", [128, 128], dt.bfloat16, isOutput=False)
    p_out = nc.declare_dram_parameter("p_out", [TS, C], dt.float32, isOutput=True)

    with tile.TileContext(nc, num_cores=N_CORES) as tc:
        with (
            tc.tile_pool(name="persist", bufs=1) as pp,
            tc.tile_pool(name="dram", bufs=1, space="DRAM") as pdram,
        ):
            # Consumer-less first collective: absorbs the one-time
            # rendezvous (core-launch skew + comm init) while stage A runs.
            warm_in = pdram.tile([1, 16], dt.bfloat16)
            warm_out = pdram.tile([N_CORES, 1, 16], dt.bfloat16)
            nc.gpsimd.collective_compute(
                "AllGather", alu.bypass,
                replica_groups=[list(range(N_CORES))],
                ins=[warm_in.opt()],
                outs=[warm_out.opt()],
            )

            # ------------- persistent constants & activation tensors -------------
            # (tiles declared here; their DMAs are issued after the x^T/weight
            # DMAs so the first QKV matmul isn't stuck behind ~15 tiny loads)
            ident = pp.tile([128, 128], dt.bfloat16)
            maskw = pp.tile([128, 4, 512], dt.bfloat16)
            ones128_row = pp.tile([1, 128], dt.bfloat16)
            nc.vector.memset(ones128_row[:], 1.0)
            cq = pp.tile([2, HD2], dt.bfloat16)
            ck = pp.tile([2, HD2], dt.bfloat16)
            cv = pp.tile([2, HD2], dt.bfloat16)


            qT = pp.tile([128, TT], dt.bfloat16)
            kT = pp.tile([128, TT], dt.bfloat16)
            v = pp.tile([128, NT, 2, 65], dt.bfloat16)
            ctxT = pp.tile([128, TT], dt.bfloat16)

            # ---------------- stage A: LN1 stats (sharded) + QKV ----------------
            with (
                tc.tile_pool(name="xtpool", bufs=1) as pxt,
                tc.tile_pool(name="xin", bufs=4) as px,
                tc.tile_pool(name="stat", bufs=1) as pst,
                tc.tile_pool(name="apsum", bufs=3, space="PSUM") as pps_a,
                tc.tile_pool(name="apsum1", bufs=1, space="PSUM") as pps_a1,
            ):
                # x^T chunks 0-1 + QKV weights lead the DMA queue so the
                # first matmuls can start ~3us in.
                xT = pxt.tile([128, NKC, TT], dt.bfloat16)
                for ch in range(4):
                    nc.sync.dma_start(
                        xT[:, :, 512 * ch:512 * (ch + 1)],
                        p_xT.ap()[:, 512 * ch:512 * (ch + 1)].rearrange(
                            "(k p) t -> p k t", p=128))
                wq = pst.tile([128, NKC, HD2], dt.bfloat16)
                nc.sync.dma_start(wq[:], p_wq.ap().rearrange("(k p) h -> p k h", p=128))
                wk = pst.tile([128, NKC, HD2], dt.bfloat16)
                nc.sync.dma_start(wk[:], p_wk.ap().rearrange("(k p) h -> p k h", p=128))
                wv = pst.tile([128, NKC, HD2], dt.bfloat16)
                nc.sync.dma_start(wv[:], p_wv.ap().rearrange("(k p) h -> p k h", p=128))
                nc.sync.dma_start(ident[:], p_ident[:])
                nc.sync.dma_start(maskw[:], p_maskw.ap().rearrange(
                    "p (o t) -> p o t", o=4))
                nc.sync.dma_start(cq[:], p_cq[:])
                nc.sync.dma_start(ck[:], p_ck[:])
                nc.sync.dma_start(cv[:], p_cv[:])

                # rows_all [2, TT]: row 0 = -mu, row 1 = std+eps
                rows_all = pst.tile([2, TT], dt.bfloat16)
                inv_row = pst.tile([1, TT], dt.bfloat16)
                inv_b = pst.tile([128, TT], dt.bfloat16)
                # LN1 stats for ALL tokens, computed redundantly per core
                # (no collective; the AllToAll is the only sync point)
                for sg in range(8):
                    # interleave the remaining x^T chunks with the stats x
                    # tiles so both DMA streams progress together
                    ch = sg + 4
                    if ch < TT // 512:
                        nc.sync.dma_start(
                            xT[:, :, 512 * ch:512 * (ch + 1)],
                            p_xT.ap()[:, 512 * ch:512 * (ch + 1)].rearrange(
                                "(k p) t -> p k t", p=128))
                    stats = px.tile([128, 4, 2], dt.float32, tag="stats")
                    for i in range(4):
                        ti = 4 * sg + i
                        xt = px.tile([128, C], dt.bfloat16, tag="xtc", bufs=8)
                        nc.sync.dma_start(xt[:], p_x[128 * ti:128 * (ti + 1), :])
                        bnt = px.tile([128, 2, 6], dt.float32, tag="bnt")
                        nc.vector.bn_stats(bnt[:, 0, :], xt[:, 0:512])
                        nc.vector.bn_stats(bnt[:, 1, :], xt[:, 512:1024])
                        nc.vector.bn_aggr(stats[:, i, :], bnt[:])
                    stat3 = px.tile([128, 4, 3], dt.bfloat16, tag="stat3")
                    stdf = px.tile([128, 4], dt.float32, tag="stdf")
                    nc.scalar.activation(stdf[:], stats[:, :, 1], act.Sqrt,
                                         scale=float(C) / (C - 1))
                    nc.vector.tensor_scalar(stdf[:], stdf[:], EPS, None, alu.add)
                    invf2 = px.tile([128, 4], dt.float32, tag="invf")
                    nc.vector.reciprocal(invf2[:], stdf[:])
                    nc.vector.tensor_scalar(stat3[:, :, 0], stats[:, :, 0],
                                            -1.0, None, alu.mult)
                    nc.vector.tensor_copy(stat3[:, :, 1], stdf[:])
                    nc.vector.tensor_copy(stat3[:, :, 2], invf2[:])
                    for i in range(4):
                        col = 512 * sg + 128 * i
                        pt = pps_a1.tile([2, 128], dt.bfloat16, tag="rowtp")
                        nc.tensor.transpose(pt[:], stat3[:, i, 0:2], ident[:])
                        nc.scalar.copy(rows_all[:, col:col + 128], pt[:])
                        ptv = pps_a1.tile([1, 128], dt.bfloat16, tag="rowtp")
                        nc.tensor.transpose(ptv[:], stat3[:, i, 2:3], ident[:])
                        nc.scalar.copy(inv_row[:, col:col + 128], ptv[:])

                # main QKV matmuls, independent of the LN1 stats exchange:
                # raw results parked in bf16, corrected once stats arrive.
                vT = pxt.tile([128, TT], dt.bfloat16)
                for cp in range(TT // 1024):
                    sls = [slice(1024 * cp, 1024 * cp + 512),
                           slice(1024 * cp + 512, 1024 * (cp + 1))]
                    for (nm, w, cw, dst) in (("q", wq, cq, qT), ("k", wk, ck, kT),
                                             ("v", wv, cv, vT)):
                        pss = [pps_a.tile([128, 512], dt.float32,
                                          name=f"ps{nm}{i}", tag="qkv", bufs=4)
                               for i in range(2)]
                        for k in range(NKC):
                            for i in range(2):
                                nc.tensor.matmul(pss[i][:], w[:, k, :],
                                                 xT[:, k, sls[i]],
                                                 start=(k == 0),
                                                 stop=(k == NKC - 1))
                        for i in range(2):
                            nc.scalar.copy(dst[:, sls[i]], pss[i][:])

                for ch in range(TT // 512):
                    pb = pps_a1.tile([128, 512], dt.float32, tag="rowtp")
                    nc.tensor.matmul(pb[:], ones128_row[:],
                                     inv_row[0:1, 512 * ch:512 * (ch + 1)],
                                     start=True, stop=True)
                    nc.scalar.copy(inv_b[:, 512 * ch:512 * (ch + 1)], pb[:])

                # rank-2 correction + 1/std scaling
                for ch in range(TT // 512):
                    sl = slice(512 * ch, 512 * (ch + 1))
                    for (nm, w, cw, dst) in (("q", wq, cq, qT), ("k", wk, ck, kT),
                                             ("v", wv, cv, vT)):
                        pc2 = pps_a.tile([128, 512], dt.float32,
                                         name=f"pc2{nm}", tag="corr", bufs=2)
                        nc.tensor.matmul(pc2[:], cw[:], rows_all[0:2, sl],
                                         start=True, stop=True)
                        # keep the add/mult off the vector engine: it is the
                        # bn_stats bottleneck in this window.  Pool cannot
                        # read PSUM, so evict via scalar first.
                        c2b = px.tile([128, 512], dt.bfloat16, tag="c2b",
                                      bufs=3)
                        nc.scalar.copy(c2b[:], pc2[:])
                        t1 = px.tile([128, 512], dt.bfloat16, tag="t1", bufs=3)
                        nc.gpsimd.tensor_tensor(t1[:], dst[:, sl], c2b[:],
                                                alu.add)
                        eng = nc.gpsimd if nm == "v" else nc.vector
                        eng.tensor_tensor(dst[:, sl], t1[:], inv_b[:, sl],
                                          alu.mult)

                # v_aug [s, tile, head, 65] via PE transpose of vT; col 64 = 1
                nc.vector.memset(v[:, :, :, 64], 1.0)
                for i in range(NT):
                    pvt = pps_a1.tile([128, 128], dt.bfloat16, tag="vtp")
                    nc.tensor.transpose(pvt[:], vT[:, 128 * i:128 * (i + 1)],
                                        ident[:])
                    nc.scalar.copy(v[:, i, :, 0:64],
                                   pvt[:].rearrange("p (h d) -> p h d", h=2))

            # -------- prefetch stage-C weights (overlaps attention) --------
            womov = pp.tile([128, NKC, C], dt.bfloat16)
            nc.sync.dma_start(womov[:],
                              p_womov.ap().rearrange("p (k c) -> p k c", k=NKC))
            xs = pp.tile([128, 4, C], dt.float32)
            nc.sync.dma_start(xs[:],
                              p_xs.ap().rearrange("(tt p) c -> p tt c", p=128))
            b2r = pp.tile([1, C], dt.bfloat16)
            nc.sync.dma_start(b2r[:], p_b2[:])
            b1c = pp.tile([128, NMF], dt.float32)
            nc.sync.dma_start(b1c[:], p_b1c[:])

            cc_in = pdram.tile([N_CORES, 128, TS], dt.bfloat16)
            cc_out = pdram.tile([N_CORES, 128, TS], dt.bfloat16)

            # ---------------- stage B: attention ----------------
            # indicator [2,128]: row h -> partitions 64h..64h+63
            ind2 = pp.tile([2, 128], dt.bfloat16)
            nc.sync.dma_start(ind2[:], p_ind2[:])
            # Scores for both heads share one [128,1024] PSUM tile (2 banks)
            # -> one exp per j.  Z-normalization is deferred: raw ctx + z rows
            # are evicted per group, one batched reciprocal per batch half,
            # normalize overlapped with the next batch's scores.
            with (
                tc.tile_pool(name="exps", bufs=8) as pexp,
                tc.tile_pool(name="attsb", bufs=2) as pat,
                tc.tile_pool(name="battn", bufs=1) as pbat,
                tc.tile_pool(name="scpsum", bufs=3, space="PSUM") as pps_sc,
                tc.tile_pool(name="ctxpsum", bufs=1, space="PSUM") as pps_ctx,
            ):
                ctxu = pbat.tile([128, TT], dt.bfloat16)  # unnormalized ctx
                # z rows: batch b at partitions 32b..32b+7 (32-aligned reads)
                ZROW = {g: 32 * (g // 4) + 2 * (g % 4) for g in range(8)}
                zall = pbat.tile([40, 512], dt.float32)
                zinvf = pbat.tile([40, 512], dt.float32)
                zinv16 = pbat.tile([40, 512], dt.bfloat16)

                def group_epilogue(b):
                    # 1/z for this batch's 8 (group, head) rows, then scale.
                    # Batch 1 runs after all scores -> may reuse the sc psum;
                    # batch 0 must not disturb the live score pipeline.
                    r0 = 32 * b
                    tag, pool = "sc", pps_sc
                    nc.vector.reciprocal(zinvf[r0:r0 + 8, :],
                                         zall[r0:r0 + 8, :])
                    nc.vector.tensor_copy(zinv16[r0:r0 + 8, :],
                                          zinvf[r0:r0 + 8, :])
                    zgbs = []
                    for qt in range(4):
                        zgb = pat.tile([2, 512], dt.bfloat16, tag="zgb",
                                       bufs=4)
                        nc.sync.dma_start(
                            zgb[:], zinv16[r0 + 2 * qt:r0 + 2 * qt + 2, :])
                        zgbs.append(zgb)
                    for qt in range(4):
                        g = b * 4 + qt
                        gsl = slice(512 * g, 512 * (g + 1))
                        pzb = pool.tile([128, 512], dt.float32, tag=tag,
                                        name="pzb")
                        nc.tensor.matmul(pzb[:], ind2[:], zgbs[qt][:],
                                         start=True, stop=True)
                        zb = pat.tile([128, 512], dt.bfloat16, tag="zbs")
                        nc.vector.tensor_copy(zb[:], pzb[:])
                        nc.vector.tensor_tensor(ctxT[:, gsl], ctxu[:, gsl],
                                                zb[:], alu.mult)
                        nc.sync.dma_start(cc_in[g], ctxT[:, gsl])

                for b in range(B):
                    for qt in range(T // 512):
                        G = b * T + 512 * qt
                        g = b * 4 + qt
                        gsl = slice(G, G + 512)
                        nj = 4 * qt + 4
                        pc = [pps_ctx.tile([65, 512], dt.float32,
                                           name=f"pc{h}", tag=f"ctx{h}")
                              for h in range(2)]
                        ets = []
                        for j in range(nj):
                            st = b * (T // 128) + j   # global s-tile index
                            ps = pps_sc.tile([128, 1024], dt.float32, tag="sc",
                                             name="ps")
                            for h in range(2):
                                hsl = slice(64 * h, 64 * (h + 1))
                                nc.tensor.matmul(
                                    ps[:, 512 * h:512 * (h + 1)],
                                    kT[hsl, 128 * st:128 * (st + 1)],
                                    qT[hsl, gsl], start=True, stop=True)
                            et = pexp.tile([128, 1024], dt.bfloat16, tag="et",
                                           name="et")
                            if j >= nj - 4:
                                off = j - (nj - 4)
                                if off > 0:
                                    nc.gpsimd.memset(
                                        et[:].rearrange("p (h t) -> p h t",
                                                        h=2)[:, :, 0:128 * off],
                                        0.0)
                                for h in range(2):
                                    o = 512 * h + 128 * off
                                    nc.scalar.activation(
                                        et[:, o:512 * (h + 1)],
                                        ps[:, o:512 * (h + 1)],
                                        act.Exp, scale=1.0 / float(np.sqrt(H)))
                                    nc.vector.tensor_tensor(
                                        et[:, o:o + 128], et[:, o:o + 128],
                                        maskw[:, off, 128 * off:128 * (off + 1)],
                                        alu.mult)
                            else:
                                nc.scalar.activation(et[:], ps[:], act.Exp,
                                                     scale=1.0 / float(np.sqrt(H)))
                            ets.append(et)
                            # software pipeline: AV for tile j-2 after scores of j
                            if j >= 2:
                                for h in range(2):
                                    nc.tensor.matmul(
                                        pc[h][:], v[:, b * (T // 128) + j - 2, h, :],
                                        ets[j - 2][:, 512 * h:512 * (h + 1)],
                                        start=(j - 2 == 0), stop=False)
                        for jt in (nj - 2, nj - 1):
                            for h in range(2):
                                nc.tensor.matmul(
                                    pc[h][:], v[:, b * (T // 128) + jt, h, :],
                                    ets[jt][:, 512 * h:512 * (h + 1)],
                                    start=(jt == 0), stop=(jt == nj - 1))
                        # evict raw ctx + z row; normalization is deferred
                        for h in range(2):
                            zrow = pat.tile([1, 512], dt.float32, tag="zr",
                                            bufs=4)
                            nc.vector.tensor_copy(zrow[:], pc[h][64:65, :])
                            r = ZROW[g] + h
                            nc.sync.dma_start(zall[r:r + 1, :], zrow[:])
                            nc.vector.tensor_copy(
                                ctxu[64 * h:64 * (h + 1), gsl], pc[h][0:64, :])
                        if g == 4:
                            group_epilogue(0)
                if True:
                    group_epilogue(1)

            # ---------------- AllToAll: heads -> tokens ----------------
            nc.gpsimd.collective_compute(
                "AllToAll", alu.bypass,
                replica_groups=[list(range(N_CORES))],
                ins=[cc_in.opt()],
                outs=[cc_out.opt()],
            )

            # ------- stage C: Wo + LN2 + FFN, [token, channel] layout -------
            with (
                tc.tile_pool(name="postsb", bufs=1) as pq,
                tc.tile_pool(name="wstream", bufs=2) as pw,
                tc.tile_pool(name="evict", bufs=3) as pev,
                tc.tile_pool(name="ln2", bufs=1) as pl2,
            ):
                w2mov = pq.tile([128, NMF, C], dt.bfloat16)
                nc.sync.dma_start(w2mov[:],
                                  p_w2mov.ap().rearrange("p (k c) -> p k c",
                                                         k=NMF))
                ctxF = pq.tile([128, NKC, TS], dt.bfloat16)
                for j in range(N_CORES):
                    nc.sync.dma_start(ctxF[:, j, :], cc_out[j])

                r2 = pq.tile([128, 4, C], dt.float32)     # x + bo + attn, [tok, c]
                xn2T = pq.tile([128, NKC, TS], dt.bfloat16)
                st2 = pl2.tile([128, 4, 2], dt.float32)
                stdt = pl2.tile([128, 4], dt.float32)
                inv2c = pl2.tile([128, 4], dt.float32)
                negmu = pl2.tile([128, 4], dt.float32)

                with (
                    tc.tile_pool(name="wops", bufs=4, space="PSUM") as pps_wo,
                    tc.tile_pool(name="tpps", bufs=4, space="PSUM") as pps_tp,
                ):
                    # Wo: stationary ctx blocks, moving WoT; out [tok, c]
                    for tt in range(4):
                        tsl = slice(128 * tt, 128 * (tt + 1))
                        pwos = [pps_wo.tile([128, 512], dt.float32, tag="wo",
                                            name=f"pwo{cb}")
                                for cb in range(2)]
                        for k in range(NKC):
                            for cb in range(2):
                                csl = slice(512 * cb, 512 * (cb + 1))
                                nc.tensor.matmul(pwos[cb][:], ctxF[:, k, tsl],
                                                 womov[:, k, csl],
                                                 start=(k == 0),
                                                 stop=(k == NKC - 1))
                        for cb in range(2):
                            csl = slice(512 * cb, 512 * (cb + 1))
                            nc.vector.tensor_tensor(r2[:, tt, csl], pwos[cb][:],
                                                    xs[:, tt, csl], alu.add)
                        # LN2 stats for this token tile
                        bnt = pl2.tile([128, 2, 6], dt.float32, tag="bnt",
                                       bufs=2)
                        nc.vector.bn_stats(bnt[:, 0, :], r2[:, tt, 0:512])
                        nc.vector.bn_stats(bnt[:, 1, :], r2[:, tt, 512:1024])
                        nc.vector.bn_aggr(st2[:, tt, :], bnt[:])

                    # (x - mu) / (std + eps), torch ddof=1
                    nc.scalar.activation(stdt[:], st2[:, :, 1], act.Sqrt,
                                         scale=float(C) / (C - 1))
                    nc.vector.tensor_scalar(stdt[:], stdt[:], EPS, None, alu.add)
                    nc.vector.reciprocal(inv2c[:], stdt[:])
                    nc.vector.tensor_scalar(negmu[:], st2[:, :, 0], -1.0, None,
                                            alu.mult)
                    for tt in range(4):
                        xn2 = pev.tile([128, C], dt.bfloat16, tag="xn2",
                                       bufs=2)
                        nc.vector.tensor_scalar(xn2[:], r2[:, tt, :],
                                                negmu[:, tt:tt + 1],
                                                inv2c[:, tt:tt + 1],
                                                alu.add, alu.mult)
                        # transpose to [c, tok] for FFN1
                        for kc in range(NKC):
                            pt = pps_tp.tile([128, 128], dt.bfloat16, tag="tp")
                            nc.tensor.transpose(
                                pt[:], xn2[:, 128 * kc:128 * (kc + 1)],
                                ident[:])
                            nc.scalar.copy(xn2T[:, kc, 128 * tt:128 * (tt + 1)],
                                           pt[:])

                # FFN1: stationary W1 blocks -> hT [ff, tok]
                hT = pq.tile([128, NMF, TS], dt.bfloat16)
                with tc.tile_pool(name="ffps", bufs=4, space="PSUM") as pps_ff:
                    for mf in range(NMF):
                        w1_blk = pw.tile([128, NKC, 128], dt.bfloat16, tag="w1",
                                         bufs=8)
                        nc.sync.dma_start(
                            w1_blk[:],
                            p_w1blk[mf].rearrange("(k p) f -> p k f", p=128))
                        pf = pps_ff.tile([128, TS], dt.float32, tag="ff")
                        for k in range(NKC):
                            nc.tensor.matmul(pf[:], w1_blk[:, k, :],
                                             xn2T[:, k, :],
                                             start=(k == 0),
                                             stop=(k == NKC - 1))
                        nc.scalar.activation(hT[:, mf, :], pf[:], act.Relu,
                                             bias=b1c[:, mf:mf + 1])

                # FFN2: stationary hT blocks, moving W2T; out [tok, c]
                with tc.tile_pool(name="ff2ps", bufs=8, space="PSUM") as pps_f2:
                    for tt in range(4):
                        tsl = slice(128 * tt, 128 * (tt + 1))
                        pos = [pps_f2.tile([128, 512], dt.float32, tag="f2",
                                           name=f"po{cb}") for cb in range(2)]
                        for k in range(NMF):
                            for cb in range(2):
                                csl = slice(512 * cb, 512 * (cb + 1))
                                nc.tensor.matmul(pos[cb][:], hT[:, k, tsl],
                                                 w2mov[:, k, csl],
                                                 start=(k == 0), stop=False)
                        for cb in range(2):
                            csl = slice(512 * cb, 512 * (cb + 1))
                            nc.tensor.matmul(pos[cb][:], ones128_row[:],
                                             b2r[0:1, csl],
                                             start=False, stop=True)
                            ot = pev.tile([128, 512], dt.float32, tag="ot")
                            nc.vector.tensor_tensor(ot[:], pos[cb][:],
                                                    r2[:, tt, csl], alu.add)
                            nc.sync.dma_start(p_out[tsl, csl], ot[:])

    nc.compile()
    return nc


def _host_prep(inputs):
    """Fold layernorm affine params into weights; build per-core input maps."""
    x = np.asarray(inputs["x"], np.float32)
    Wq = np.asarray(inputs["Wq"], np.float32)
    Wk = np.asarray(inputs["Wk"], np.float32)
    Wv = np.asarray(inputs["Wv"], np.float32)
    Wo = np.asarray(inputs["Wo"], np.float32)
    bo = np.asarray(inputs["bo"], np.float32)
    W1 = np.asarray(inputs["W1"], np.float32)
    b1 = np.asarray(inputs["b1"], np.float32)
    W2 = np.asarray(inputs["W2"], np.float32)
    b2 = np.asarray(inputs["b2"], np.float32)
    g1 = np.asarray(inputs["g1"], np.float32)
    be1 = np.asarray(inputs["be1"], np.float32)
    g2 = np.asarray(inputs["g2"], np.float32)
    be2 = np.asarray(inputs["be2"], np.float32)

    xf = x.reshape(TT, C)                      # both batches stacked
    xT = np.ascontiguousarray(xf.T)            # [C, TT]

    def fold_qkv(W):
        Weff = g1[:, None] * W                  # [NH, C, H] with g1 on C
        Weff = np.ascontiguousarray(np.transpose(Weff, (1, 0, 2)))  # [C, NH, H]
        bias = np.einsum("c,hck->hk", be1, W)   # [NH, H]
        colsum = Weff.sum(axis=0)               # [NH, H]
        return Weff, bias, colsum

    Wq_e, bq, csq = fold_qkv(Wq)
    Wk_e, bk, csk = fold_qkv(Wk)
    Wv_e, bv, csv = fold_qkv(Wv)

    woT = np.ascontiguousarray(Wo.T)            # [NH*H, C]
    w1T = np.ascontiguousarray(g2[:, None] * W1.T)   # [C, FF]
    b1_eff = b1 + be2 @ W1.T                         # [FF]
    w2T = np.ascontiguousarray(W2.T)            # [FF, C]

    w1blk = np.ascontiguousarray(
        w1T.reshape(C, NMF, 128).transpose(1, 0, 2))
    womov = np.ascontiguousarray(
        woT.reshape(NKC, 128, C).transpose(1, 0, 2)).reshape(128, NKC * C)
    w2mov = np.ascontiguousarray(
        w2T.reshape(NMF, 128, C).transpose(1, 0, 2)).reshape(128, NMF * C)

    tq = np.arange(128)[None, :]
    s = np.arange(128)[:, None]
    maskd = (s <= tq).astype(np.float32)
    maskw = np.zeros((128, 4, 512), np.float32)
    for off in range(4):
        maskw[:, off, 128 * off:128 * (off + 1)] = maskd

    x_bf = xf.astype(BF16)
    shared = {
        "p_x": x_bf,
        "p_xT": xT.astype(BF16),
        "p_womov": womov.astype(BF16),
        "p_w1blk": w1blk.astype(BF16),
        "p_b1c": np.ascontiguousarray(
            b1_eff.reshape(NMF, 128).T).astype(np.float32),
        "p_w2mov": w2mov.astype(BF16),
        "p_b2": b2[None, :].astype(BF16),
        "p_maskw": maskw.reshape(128, 4 * 512).astype(BF16),
        "p_ind2": np.repeat(np.eye(2, dtype=np.float32), 64, axis=1).astype(BF16),
        "p_ident": np.eye(128, dtype=np.float32).astype(BF16),
    }

    in_maps = []
    for r in range(N_CORES):
        h0 = HPC * r
        hs = slice(h0, h0 + HPC)
        b_r, s_r = divmod(r, N_CORES // B)
        tok = slice(s_r * TS, (s_r + 1) * TS)
        m = dict(shared)
        m["p_xs"] = (x[b_r][tok, :] + bo[None, :]).astype(np.float32)
        m["p_wq"] = np.ascontiguousarray(
            Wq_e[:, hs, :].reshape(C, HD2)).astype(BF16)
        m["p_wk"] = np.ascontiguousarray(
            Wk_e[:, hs, :].reshape(C, HD2)).astype(BF16)
        m["p_wv"] = np.ascontiguousarray(
            Wv_e[:, hs, :].reshape(C, HD2)).astype(BF16)
        m["p_cq"] = np.stack([csq[hs].reshape(HD2),
                              bq[hs].reshape(HD2)]).astype(BF16)
        m["p_ck"] = np.stack([csk[hs].reshape(HD2),
                              bk[hs].reshape(HD2)]).astype(BF16)
        m["p_cv"] = np.stack([csv[hs].reshape(HD2),
                              bv[hs].reshape(HD2)]).astype(BF16)
        in_maps.append(m)
    return in_maps


def kernel(**inputs) -> np.ndarray:
    from concourse.bass_utils import run_bass_kernel_spmd

    if "nc" not in _BUILT:
        _BUILT["nc"] = _build()
    nc = _BUILT["nc"]

    in_maps = _host_prep(inputs)
    res = run_bass_kernel_spmd(nc, in_maps, core_ids=list(range(N_CORES)))

    out = np.empty((B, T, C), np.float32)
    for r in range(N_CORES):
        b_r, s_r = divmod(r, N_CORES // B)
        out[b_r, s_r * TS:(s_r + 1) * TS, :] = res.results[r]["p_out"]
    return out



# revision 45
# speedup vs baseline: 1.2234x; 1.2234x over previous
"""Trainium2 Bass kernel for a dense transformer block (pre-LN, 16-head causal
attention + 3x FFN), distributed over 8 NeuronCores.

Sharding: tensor-parallel over heads (2 heads/core, both batch elements on
every core) for LN1/QKV/attention; one 8-core AllToAll redistributes the
per-head attention context to token-parallel shards (512 tokens/core) for the
output projection, LN2 and the FFN.  Matmuls run in bf16 with f32 PSUM
accumulation; the residual stream stays f32.

All layouts are transposed ([channel, token]) on chip so every matmul
contracts over the partition dim.  LayerNorm 1 is folded into the QKV weights:
q = inv_std[t] * (x @ Wq_eff - mu[t] * colsum(Wq_eff)) + be1 @ Wq, implemented
with a rank-2 correction matmul appended to each accumulation group.
"""

import numpy as np
import ml_dtypes

B, T, C = 2, 2048, 1024
NH, H = 16, 64
FF = 3 * C
EPS = 1e-6
N_CORES = 8
TT = B * T            # 4096 tokens processed per core (head-parallel phase)
TS = TT // N_CORES    # 512 tokens per core (token-parallel phase)
HPC = NH // N_CORES   # 2 heads per core
HD2 = HPC * H         # 128

BF16 = ml_dtypes.bfloat16

_BUILT = {}

NT = TT // 128        # 32 token tiles
NKC = C // 128        # 8 channel k-tiles
NMF = FF // 128       # 24 ff tiles


def _build():
    import concourse.bacc as bacc
    import concourse.mybir as mybir
    import concourse.tile as tile
    dt = mybir.dt
    alu = mybir.AluOpType
    act = mybir.ActivationFunctionType

    nc = bacc.Bacc("TRN2", target_bir_lowering=False, debug=False,
                   num_devices=N_CORES)

    # ----- kernel I/O (per-core shards) -----
    p_x = nc.declare_dram_parameter("p_x", [TT, C], dt.bfloat16, isOutput=False)
    p_xT = nc.declare_dram_parameter("p_xT", [C, TT], dt.bfloat16, isOutput=False)
    p_xs = nc.declare_dram_parameter("p_xs", [TS, C], dt.float32, isOutput=False)
    p_wq = nc.declare_dram_parameter("p_wq", [C, HD2], dt.bfloat16, isOutput=False)
    p_wk = nc.declare_dram_parameter("p_wk", [C, HD2], dt.bfloat16, isOutput=False)
    p_wv = nc.declare_dram_parameter("p_wv", [C, HD2], dt.bfloat16, isOutput=False)
    p_cq = nc.declare_dram_parameter("p_cq", [2, HD2], dt.bfloat16, isOutput=False)
    p_ck = nc.declare_dram_parameter("p_ck", [2, HD2], dt.bfloat16, isOutput=False)
    p_cv = nc.declare_dram_parameter("p_cv", [2, HD2], dt.bfloat16, isOutput=False)
    p_womov = nc.declare_dram_parameter("p_womov", [128, NKC * C], dt.bfloat16, isOutput=False)
    p_w1blk = nc.declare_dram_parameter("p_w1blk", [NMF, C, 128], dt.bfloat16, isOutput=False)
    p_b1c = nc.declare_dram_parameter("p_b1c", [128, NMF], dt.float32, isOutput=False)
    p_w2mov = nc.declare_dram_parameter("p_w2mov", [128, NMF * C], dt.bfloat16, isOutput=False)
    p_b2 = nc.declare_dram_parameter("p_b2", [1, C], dt.bfloat16, isOutput=False)
    p_maskw = nc.declare_dram_parameter("p_maskw", [128, 4 * 512], dt.bfloat16, isOutput=False)
    p_ind2 = nc.declare_dram_parameter("p_ind2", [2, 128], dt.bfloat16, isOutput=False)
    p_ident = nc.declare_dram_parameter("p_ident", [128, 128], dt.bfloat16, isOutput=False)
    p_out = nc.declare_dram_parameter("p_out", [TS, C], dt.float32, isOutput=True)

    with tile.TileContext(nc, num_cores=N_CORES) as tc:
        with (
            tc.tile_pool(name="persist", bufs=1) as pp,
            tc.tile_pool(name="dram", bufs=1, space="DRAM") as pdram,
        ):
            # Consumer-less first collective: absorbs the one-time
            # rendezvous (core-launch skew + comm init) while stage A runs.
            warm_in = pdram.tile([1, 16], dt.bfloat16)
            warm_out = pdram.tile([N_CORES, 1, 16], dt.bfloat16)
            nc.gpsimd.collective_compute(
                "AllGather", alu.bypass,
                replica_groups=[list(range(N_CORES))],
                ins=[warm_in.opt()],
                outs=[warm_out.opt()],
            )

            # ------------- persistent constants & activation tensors -------------
            # (tiles declared here; their DMAs are issued after the x^T/weight
            # DMAs so the first QKV matmul isn't stuck behind ~15 tiny loads)
            ident = pp.tile([128, 128], dt.bfloat16)
            maskw = pp.tile([128, 4, 512], dt.bfloat16)
            ones128_row = pp.tile([1, 128], dt.bfloat16)
            nc.vector.memset(ones128_row[:], 1.0)
            cq = pp.tile([2, HD2], dt.bfloat16)
            ck = pp.tile([2, HD2], dt.bfloat16)
            cv = pp.tile([2, HD2], dt.bfloat16)


            qT = pp.tile([128, TT], dt.bfloat16)
            kT = pp.tile([128, TT], dt.bfloat16)
            v = pp.tile([128, NT, 2, 65], dt.bfloat16)
            ctxT = pp.tile([128, TT], dt.bfloat16)

            # ---------------- stage A: LN1 stats (sharded) + QKV ----------------
            with (
                tc.tile_pool(name="xtpool", bufs=1) as pxt,
                tc.tile_pool(name="xin", bufs=4) as px,
                tc.tile_pool(name="stat", bufs=1) as pst,
                tc.tile_pool(name="apsum", bufs=3, space="PSUM") as pps_a,
                tc.tile_pool(name="apsum1", bufs=1, space="PSUM") as pps_a1,
            ):
                # x^T chunks 0-1 + QKV weights lead the DMA queue so the
                # first matmuls can start ~3us in.
                xT = pxt.tile([128, NKC, TT], dt.bfloat16)
                for ch in range(4):
                    nc.sync.dma_start(
                        xT[:, :, 512 * ch:512 * (ch + 1)],
                        p_xT.ap()[:, 512 * ch:512 * (ch + 1)].rearrange(
                            "(k p) t -> p k t", p=128))
                wq = pst.tile([128, NKC, HD2], dt.bfloat16)
                nc.sync.dma_start(wq[:], p_wq.ap().rearrange("(k p) h -> p k h", p=128))
                wk = pst.tile([128, NKC, HD2], dt.bfloat16)
                nc.sync.dma_start(wk[:], p_wk.ap().rearrange("(k p) h -> p k h", p=128))
                wv = pst.tile([128, NKC, HD2], dt.bfloat16)
                nc.sync.dma_start(wv[:], p_wv.ap().rearrange("(k p) h -> p k h", p=128))
                nc.sync.dma_start(ident[:], p_ident[:])
                nc.sync.dma_start(maskw[:], p_maskw.ap().rearrange(
                    "p (o t) -> p o t", o=4))
                nc.sync.dma_start(cq[:], p_cq[:])
                nc.sync.dma_start(ck[:], p_ck[:])
                nc.sync.dma_start(cv[:], p_cv[:])

                # rows_all [2, TT]: row 0 = -mu, row 1 = std+eps
                rows_all = pst.tile([2, TT], dt.bfloat16)
                inv_row = pst.tile([1, TT], dt.bfloat16)
                inv_b = pst.tile([128, TT], dt.bfloat16)
                # LN1 stats for ALL tokens, computed redundantly per core
                # (no collective; the AllToAll is the only sync point)
                for sg in range(8):
                    # interleave the remaining x^T chunks with the stats x
                    # tiles so both DMA streams progress together
                    ch = sg + 4
                    if ch < TT // 512:
                        nc.sync.dma_start(
                            xT[:, :, 512 * ch:512 * (ch + 1)],
                            p_xT.ap()[:, 512 * ch:512 * (ch + 1)].rearrange(
                                "(k p) t -> p k t", p=128))
                    stats = px.tile([128, 4, 2], dt.float32, tag="stats")
                    for i in range(4):
                        ti = 4 * sg + i
                        xt = px.tile([128, C], dt.bfloat16, tag="xtc", bufs=8)
                        nc.sync.dma_start(xt[:], p_x[128 * ti:128 * (ti + 1), :])
                        bnt = px.tile([128, 2, 6], dt.float32, tag="bnt")
                        nc.vector.bn_stats(bnt[:, 0, :], xt[:, 0:512])
                        nc.vector.bn_stats(bnt[:, 1, :], xt[:, 512:1024])
                        nc.vector.bn_aggr(stats[:, i, :], bnt[:])
                    stat3 = px.tile([128, 4, 3], dt.bfloat16, tag="stat3")
                    stdf = px.tile([128, 4], dt.float32, tag="stdf")
                    nc.scalar.activation(stdf[:], stats[:, :, 1], act.Sqrt,
                                         scale=float(C) / (C - 1))
                    nc.vector.tensor_scalar(stdf[:], stdf[:], EPS, None, alu.add)
                    invf2 = px.tile([128, 4], dt.float32, tag="invf")
                    nc.vector.reciprocal(invf2[:], stdf[:])
                    nc.vector.tensor_scalar(stat3[:, :, 0], stats[:, :, 0],
                                            -1.0, None, alu.mult)
                    nc.vector.tensor_copy(stat3[:, :, 1], stdf[:])
                    nc.vector.tensor_copy(stat3[:, :, 2], invf2[:])
                    for i in range(4):
                        col = 512 * sg + 128 * i
                        pt = pps_a1.tile([2, 128], dt.bfloat16, tag="rowtp")
                        nc.tensor.transpose(pt[:], stat3[:, i, 0:2], ident[:])
                        nc.scalar.copy(rows_all[:, col:col + 128], pt[:])
                        ptv = pps_a1.tile([1, 128], dt.bfloat16, tag="rowtp")
                        nc.tensor.transpose(ptv[:], stat3[:, i, 2:3], ident[:])
                        nc.scalar.copy(inv_row[:, col:col + 128], ptv[:])

                # main QKV matmuls, independent of the LN1 stats exchange:
                # raw results parked in bf16, corrected once stats arrive.
                vT = pxt.tile([128, TT], dt.bfloat16)
                for cp in range(TT // 1024):
                    sls = [slice(1024 * cp, 1024 * cp + 512),
                           slice(1024 * cp + 512, 1024 * (cp + 1))]
                    for (nm, w, cw, dst) in (("q", wq, cq, qT), ("k", wk, ck, kT),
                                             ("v", wv, cv, vT)):
                        pss = [pps_a.tile([128, 512], dt.float32,
                                          name=f"ps{nm}{i}", tag="qkv", bufs=4)
                               for i in range(2)]
                        for k in range(NKC):
                            for i in range(2):
                                nc.tensor.matmul(pss[i][:], w[:, k, :],
                                                 xT[:, k, sls[i]],
                                                 start=(k == 0),
                                                 stop=(k == NKC - 1))
                        for i in range(2):
                            nc.scalar.copy(dst[:, sls[i]], pss[i][:])

                for ch in range(TT // 512):
                    pb = pps_a1.tile([128, 512], dt.float32, tag="rowtp")
                    nc.tensor.matmul(pb[:], ones128_row[:],
                                     inv_row[0:1, 512 * ch:512 * (ch + 1)],
                                     start=True, stop=True)
                    nc.scalar.copy(inv_b[:, 512 * ch:512 * (ch + 1)], pb[:])

                # rank-2 correction + 1/std scaling
                for ch in range(TT // 512):
                    sl = slice(512 * ch, 512 * (ch + 1))
                    for (nm, w, cw, dst) in (("q", wq, cq, qT), ("k", wk, ck, kT),
                                             ("v", wv, cv, vT)):
                        pc2 = pps_a.tile([128, 512], dt.float32,
                                         name=f"pc2{nm}", tag="corr", bufs=2)
                        nc.tensor.matmul(pc2[:], cw[:], rows_all[0:2, sl],
                                         start=True, stop=True)
                        t1 = px.tile([128, 512], dt.bfloat16, tag="t1", bufs=3)
                        nc.vector.tensor_tensor(t1[:], dst[:, sl], pc2[:],
                                                alu.add)
                        eng = nc.gpsimd if nm == "v" else nc.vector
                        eng.tensor_tensor(dst[:, sl], t1[:], inv_b[:, sl],
                                          alu.mult)

                # v_aug [s, tile, head, 65] via PE transpose of vT; col 64 = 1
                nc.vector.memset(v[:, :, :, 64], 1.0)
                for i in range(NT):
                    pvt = pps_a1.tile([128, 128], dt.bfloat16, tag="vtp")
                    nc.tensor.transpose(pvt[:], vT[:, 128 * i:128 * (i + 1)],
                                        ident[:])
                    nc.scalar.copy(v[:, i, :, 0:64],
                                   pvt[:].rearrange("p (h d) -> p h d", h=2))

            # -------- prefetch stage-C weights (overlaps attention) --------
            womov = pp.tile([128, NKC, C], dt.bfloat16)
            nc.sync.dma_start(womov[:],
                              p_womov.ap().rearrange("p (k c) -> p k c", k=NKC))
            xs = pp.tile([128, 4, C], dt.float32)
            nc.sync.dma_start(xs[:],
                              p_xs.ap().rearrange("(tt p) c -> p tt c", p=128))
            b2r = pp.tile([1, C], dt.bfloat16)
            nc.sync.dma_start(b2r[:], p_b2[:])
            b1c = pp.tile([128, NMF], dt.float32)
            nc.sync.dma_start(b1c[:], p_b1c[:])

            cc_in = pdram.tile([N_CORES, 128, TS], dt.bfloat16)
            cc_out = pdram.tile([N_CORES, 128, TS], dt.bfloat16)

            # ---------------- stage B: attention ----------------
            # indicator [2,128]: row h -> partitions 64h..64h+63
            ind2 = pp.tile([2, 128], dt.bfloat16)
            nc.sync.dma_start(ind2[:], p_ind2[:])
            # Scores for both heads share one [128,1024] PSUM tile (2 banks)
            # -> one exp per j.  Z-normalization is deferred: raw ctx + z rows
            # are evicted per group, one batched reciprocal per batch half,
            # normalize overlapped with the next batch's scores.
            with (
                tc.tile_pool(name="exps", bufs=8) as pexp,
                tc.tile_pool(name="attsb", bufs=2) as pat,
                tc.tile_pool(name="battn", bufs=1) as pbat,
                tc.tile_pool(name="scpsum", bufs=3, space="PSUM") as pps_sc,
                tc.tile_pool(name="ctxpsum", bufs=1, space="PSUM") as pps_ctx,
            ):
                ctxu = pbat.tile([128, TT], dt.bfloat16)  # unnormalized ctx
                # z rows: batch b at partitions 32b..32b+7 (32-aligned reads)
                ZROW = {g: 32 * (g // 4) + 2 * (g % 4) for g in range(8)}
                zall = pbat.tile([40, 512], dt.float32)
                zinvf = pbat.tile([40, 512], dt.float32)
                zinv16 = pbat.tile([40, 512], dt.bfloat16)

                def group_epilogue(b):
                    # 1/z for this batch's 8 (group, head) rows, then scale.
                    # Batch 1 runs after all scores -> may reuse the sc psum;
                    # batch 0 must not disturb the live score pipeline.
                    r0 = 32 * b
                    tag, pool = "sc", pps_sc
                    nc.vector.reciprocal(zinvf[r0:r0 + 8, :],
                                         zall[r0:r0 + 8, :])
                    nc.vector.tensor_copy(zinv16[r0:r0 + 8, :],
                                          zinvf[r0:r0 + 8, :])
                    zgbs = []
                    for qt in range(4):
                        zgb = pat.tile([2, 512], dt.bfloat16, tag="zgb",
                                       bufs=4)
                        nc.sync.dma_start(
                            zgb[:], zinv16[r0 + 2 * qt:r0 + 2 * qt + 2, :])
                        zgbs.append(zgb)
                    for qt in range(4):
                        g = b * 4 + qt
                        gsl = slice(512 * g, 512 * (g + 1))
                        pzb = pool.tile([128, 512], dt.float32, tag=tag,
                                        name="pzb")
                        nc.tensor.matmul(pzb[:], ind2[:], zgbs[qt][:],
                                         start=True, stop=True)
                        zb = pat.tile([128, 512], dt.bfloat16, tag="zbs")
                        nc.vector.tensor_copy(zb[:], pzb[:])
                        nc.vector.tensor_tensor(ctxT[:, gsl], ctxu[:, gsl],
                                                zb[:], alu.mult)
                        nc.sync.dma_start(cc_in[g], ctxT[:, gsl])

                for b in range(B):
                    for qt in range(T // 512):
                        G = b * T + 512 * qt
                        g = b * 4 + qt
                        gsl = slice(G, G + 512)
                        nj = 4 * qt + 4
                        pc = [pps_ctx.tile([65, 512], dt.float32,
                                           name=f"pc{h}", tag=f"ctx{h}")
                              for h in range(2)]
                        ets = []
                        for j in range(nj):
                            st = b * (T // 128) + j   # global s-tile index
                            ps = pps_sc.tile([128, 1024], dt.float32, tag="sc",
                                             name="ps")
                            for h in range(2):
                                hsl = slice(64 * h, 64 * (h + 1))
                                nc.tensor.matmul(
                                    ps[:, 512 * h:512 * (h + 1)],
                                    kT[hsl, 128 * st:128 * (st + 1)],
                                    qT[hsl, gsl], start=True, stop=True)
                            et = pexp.tile([128, 1024], dt.bfloat16, tag="et",
                                           name="et")
                            if j >= nj - 4:
                                off = j - (nj - 4)
                                if off > 0:
                                    nc.gpsimd.memset(
                                        et[:].rearrange("p (h t) -> p h t",
                                                        h=2)[:, :, 0:128 * off],
                                        0.0)
                                for h in range(2):
                                    o = 512 * h + 128 * off
                                    nc.scalar.activation(
                                        et[:, o:512 * (h + 1)],
                                        ps[:, o:512 * (h + 1)],
                                        act.Exp, scale=1.0 / float(np.sqrt(H)))
                                    nc.vector.tensor_tensor(
                                        et[:, o:o + 128], et[:, o:o + 128],
                                        maskw[:, off, 128 * off:128 * (off + 1)],
                                        alu.mult)
                            else:
                                nc.scalar.activation(et[:], ps[:], act.Exp,
                                                     scale=1.0 / float(np.sqrt(H)))
                            ets.append(et)
                            # software pipeline: AV for tile j-2 after scores of j
                            if j >= 2:
                                for h in range(2):
                                    nc.tensor.matmul(
                                        pc[h][:], v[:, b * (T // 128) + j - 2, h, :],
                                        ets[j - 2][:, 512 * h:512 * (h + 1)],
                                        start=(j - 2 == 0), stop=False)
                        for jt in (nj - 2, nj - 1):
                            for h in range(2):
                                nc.tensor.matmul(
                                    pc[h][:], v[:, b * (T // 128) + jt, h, :],
                                    ets[jt][:, 512 * h:512 * (h + 1)],
                                    start=(jt == 0), stop=(jt == nj - 1))
                        # evict raw ctx + z row; normalization is deferred
                        for h in range(2):
                            zrow = pat.tile([1, 512], dt.float32, tag="zr",
                                            bufs=4)
                            nc.vector.tensor_copy(zrow[:], pc[h][64:65, :])
                            r = ZROW[g] + h
                            nc.sync.dma_start(zall[r:r + 1, :], zrow[:])
                            nc.vector.tensor_copy(
                                ctxu[64 * h:64 * (h + 1), gsl], pc[h][0:64, :])
                        if g == 4:
                            group_epilogue(0)
                if True:
                    group_epilogue(1)

            # ---------------- AllToAll: heads -> tokens ----------------
            nc.gpsimd.collective_compute(
                "AllToAll", alu.bypass,
                replica_groups=[list(range(N_CORES))],
                ins=[cc_in.opt()],
                outs=[cc_out.opt()],
            )

            # ------- stage C: Wo + LN2 + FFN, [token, channel] layout -------
            with (
                tc.tile_pool(name="postsb", bufs=1) as pq,
                tc.tile_pool(name="wstream", bufs=2) as pw,
                tc.tile_pool(name="evict", bufs=3) as pev,
                tc.tile_pool(name="ln2", bufs=1) as pl2,
            ):
                w2mov = pq.tile([128, NMF, C], dt.bfloat16)
                nc.sync.dma_start(w2mov[:],
                                  p_w2mov.ap().rearrange("p (k c) -> p k c",
                                                         k=NMF))
                ctxF = pq.tile([128, NKC, TS], dt.bfloat16)
                for j in range(N_CORES):
                    nc.sync.dma_start(ctxF[:, j, :], cc_out[j])

                r2 = pq.tile([128, 4, C], dt.float32)     # x + bo + attn, [tok, c]
                xn2T = pq.tile([128, NKC, TS], dt.bfloat16)
                st2 = pl2.tile([128, 4, 2], dt.float32)
                stdt = pl2.tile([128, 4], dt.float32)
                inv2c = pl2.tile([128, 4], dt.float32)
                negmu = pl2.tile([128, 4], dt.float32)

                with (
                    tc.tile_pool(name="wops", bufs=4, space="PSUM") as pps_wo,
                    tc.tile_pool(name="tpps", bufs=4, space="PSUM") as pps_tp,
                ):
                    # Wo: stationary ctx blocks, moving WoT; out [tok, c]
                    for tt in range(4):
                        tsl = slice(128 * tt, 128 * (tt + 1))
                        pwos = [pps_wo.tile([128, 512], dt.float32, tag="wo",
                                            name=f"pwo{cb}")
                                for cb in range(2)]
                        for k in range(NKC):
                            for cb in range(2):
                                csl = slice(512 * cb, 512 * (cb + 1))
                                nc.tensor.matmul(pwos[cb][:], ctxF[:, k, tsl],
                                                 womov[:, k, csl],
                                                 start=(k == 0),
                                                 stop=(k == NKC - 1))
                        for cb in range(2):
                            csl = slice(512 * cb, 512 * (cb + 1))
                            nc.vector.tensor_tensor(r2[:, tt, csl], pwos[cb][:],
                                                    xs[:, tt, csl], alu.add)
                        # LN2 stats for this token tile
                        bnt = pl2.tile([128, 2, 6], dt.float32, tag="bnt",
                                       bufs=2)
                        nc.vector.bn_stats(bnt[:, 0, :], r2[:, tt, 0:512])
                        nc.vector.bn_stats(bnt[:, 1, :], r2[:, tt, 512:1024])
                        nc.vector.bn_aggr(st2[:, tt, :], bnt[:])

                    # (x - mu) / (std + eps), torch ddof=1
                    nc.scalar.activation(stdt[:], st2[:, :, 1], act.Sqrt,
                                         scale=float(C) / (C - 1))
                    nc.vector.tensor_scalar(stdt[:], stdt[:], EPS, None, alu.add)
                    nc.vector.reciprocal(inv2c[:], stdt[:])
                    nc.vector.tensor_scalar(negmu[:], st2[:, :, 0], -1.0, None,
                                            alu.mult)
                    for tt in range(4):
                        xn2 = pev.tile([128, C], dt.bfloat16, tag="xn2",
                                       bufs=2)
                        nc.vector.tensor_scalar(xn2[:], r2[:, tt, :],
                                                negmu[:, tt:tt + 1],
                                                inv2c[:, tt:tt + 1],
                                                alu.add, alu.mult)
                        # transpose to [c, tok] for FFN1
                        for kc in range(NKC):
                            pt = pps_tp.tile([128, 128], dt.bfloat16, tag="tp")
                            nc.tensor.transpose(
                                pt[:], xn2[:, 128 * kc:128 * (kc + 1)],
                                ident[:])
                            nc.scalar.copy(xn2T[:, kc, 128 * tt:128 * (tt + 1)],
                                           pt[:])

                # FFN1: stationary W1 blocks -> hT [ff, tok]
                hT = pq.tile([128, NMF, TS], dt.bfloat16)
                with tc.tile_pool(name="ffps", bufs=4, space="PSUM") as pps_ff:
                    for mf in range(NMF):
                        w1_blk = pw.tile([128, NKC, 128], dt.bfloat16, tag="w1",
                                         bufs=8)
                        nc.sync.dma_start(
                            w1_blk[:],
                            p_w1blk[mf].rearrange("(k p) f -> p k f", p=128))
                        pf = pps_ff.tile([128, TS], dt.float32, tag="ff")
                        for k in range(NKC):
                            nc.tensor.matmul(pf[:], w1_blk[:, k, :],
                                             xn2T[:, k, :],
                                             start=(k == 0),
                                             stop=(k == NKC - 1))
                        nc.scalar.activation(hT[:, mf, :], pf[:], act.Relu,
                                             bias=b1c[:, mf:mf + 1])

                # FFN2: stationary hT blocks, moving W2T; out [tok, c]
                with tc.tile_pool(name="ff2ps", bufs=8, space="PSUM") as pps_f2:
                    for tt in range(4):
                        tsl = slice(128 * tt, 128 * (tt + 1))
                        pos = [pps_f2.tile([128, 512], dt.float32, tag="f2",
                                           name=f"po{cb}") for cb in range(2)]
                        for k in range(NMF):
                            for cb in range(2):
                                csl = slice(512 * cb, 512 * (cb + 1))
                                nc.tensor.matmul(pos[cb][:], hT[:, k, tsl],
                                                 w2mov[:, k, csl],
                                                 start=(k == 0), stop=False)
                        for cb in range(2):
                            csl = slice(512 * cb, 512 * (cb + 1))
                            nc.tensor.matmul(pos[cb][:], ones128_row[:],
                                             b2r[0:1, csl],
                                             start=False, stop=True)
                            ot = pev.tile([128, 512], dt.float32, tag="ot")
                            nc.vector.tensor_tensor(ot[:], pos[cb][:],
                                                    r2[:, tt, csl], alu.add)
                            nc.sync.dma_start(p_out[tsl, csl], ot[:])

    nc.compile()
    return nc


def _host_prep(inputs):
    """Fold layernorm affine params into weights; build per-core input maps."""
    x = np.asarray(inputs["x"], np.float32)
    Wq = np.asarray(inputs["Wq"], np.float32)
    Wk = np.asarray(inputs["Wk"], np.float32)
    Wv = np.asarray(inputs["Wv"], np.float32)
    Wo = np.asarray(inputs["Wo"], np.float32)
    bo = np.asarray(inputs["bo"], np.float32)
    W1 = np.asarray(inputs["W1"], np.float32)
    b1 = np.asarray(inputs["b1"], np.float32)
    W2 = np.asarray(inputs["W2"], np.float32)
    b2 = np.asarray(inputs["b2"], np.float32)
    g1 = np.asarray(inputs["g1"], np.float32)
    be1 = np.asarray(inputs["be1"], np.float32)
    g2 = np.asarray(inputs["g2"], np.float32)
    be2 = np.asarray(inputs["be2"], np.float32)

    xf = x.reshape(TT, C)                      # both batches stacked
    xT = np.ascontiguousarray(xf.T)            # [C, TT]

    def fold_qkv(W):
        Weff = g1[:, None] * W                  # [NH, C, H] with g1 on C
        Weff = np.ascontiguousarray(np.transpose(Weff, (1, 0, 2)))  # [C, NH, H]
        bias = np.einsum("c,hck->hk", be1, W)   # [NH, H]
        colsum = Weff.sum(axis=0)               # [NH, H]
        return Weff, bias, colsum

    Wq_e, bq, csq = fold_qkv(Wq)
    Wk_e, bk, csk = fold_qkv(Wk)
    Wv_e, bv, csv = fold_qkv(Wv)

    woT = np.ascontiguousarray(Wo.T)            # [NH*H, C]
    w1T = np.ascontiguousarray(g2[:, None] * W1.T)   # [C, FF]
    b1_eff = b1 + be2 @ W1.T                         # [FF]
    w2T = np.ascontiguousarray(W2.T)            # [FF, C]

    w1blk = np.ascontiguousarray(
        w1T.reshape(C, NMF, 128).transpose(1, 0, 2))
    womov = np.ascontiguousarray(
        woT.reshape(NKC, 128, C).transpose(1, 0, 2)).reshape(128, NKC * C)
    w2mov = np.ascontiguousarray(
        w2T.reshape(NMF, 128, C).transpose(1, 0, 2)).reshape(128, NMF * C)

    tq = np.arange(128)[None, :]
    s = np.arange(128)[:, None]
    maskd = (s <= tq).astype(np.float32)
    maskw = np.zeros((128, 4, 512), np.float32)
    for off in range(4):
        maskw[:, off, 128 * off:128 * (off + 1)] = maskd

    x_bf = xf.astype(BF16)
    shared = {
        "p_x": x_bf,
        "p_xT": xT.astype(BF16),
        "p_womov": womov.astype(BF16),
        "p_w1blk": w1blk.astype(BF16),
        "p_b1c": np.ascontiguousarray(
            b1_eff.reshape(NMF, 128).T).astype(np.float32),
        "p_w2mov": w2mov.astype(BF16),
        "p_b2": b2[None, :].astype(BF16),
        "p_maskw": maskw.reshape(128, 4 * 512).astype(BF16),
        "p_ind2": np.repeat(np.eye(2, dtype=np.float32), 64, axis=1).astype(BF16),
        "p_ident": np.eye(128, dtype=np.float32).astype(BF16),
    }

    in_maps = []
    for r in range(N_CORES):
        h0 = HPC * r
        hs = slice(h0, h0 + HPC)
        b_r, s_r = divmod(r, N_CORES // B)
        tok = slice(s_r * TS, (s_r + 1) * TS)
        m = dict(shared)
        m["p_xs"] = (x[b_r][tok, :] + bo[None, :]).astype(np.float32)
        m["p_wq"] = np.ascontiguousarray(
            Wq_e[:, hs, :].reshape(C, HD2)).astype(BF16)
        m["p_wk"] = np.ascontiguousarray(
            Wk_e[:, hs, :].reshape(C, HD2)).astype(BF16)
        m["p_wv"] = np.ascontiguousarray(
            Wv_e[:, hs, :].reshape(C, HD2)).astype(BF16)
        m["p_cq"] = np.stack([csq[hs].reshape(HD2),
                              bq[hs].reshape(HD2)]).astype(BF16)
        m["p_ck"] = np.stack([csk[hs].reshape(HD2),
                              bk[hs].reshape(HD2)]).astype(BF16)
        m["p_cv"] = np.stack([csv[hs].reshape(HD2),
                              bv[hs].reshape(HD2)]).astype(BF16)
        in_maps.append(m)
    return in_maps


def kernel(**inputs) -> np.ndarray:
    from concourse.bass_utils import run_bass_kernel_spmd

    if "nc" not in _BUILT:
        _BUILT["nc"] = _build()
    nc = _BUILT["nc"]

    in_maps = _host_prep(inputs)
    res = run_bass_kernel_spmd(nc, in_maps, core_ids=list(range(N_CORES)))

    out = np.empty((B, T, C), np.float32)
    for r in range(N_CORES):
        b_r, s_r = divmod(r, N_CORES // B)
        out[b_r, s_r * TS:(s_r + 1) * TS, :] = res.results[r]["p_out"]
    return out



# revision 46
# speedup vs baseline: 1.2238x; 1.0003x over previous
"""Trainium2 Bass kernel for a dense transformer block (pre-LN, 16-head causal
attention + 3x FFN), distributed over 8 NeuronCores.

Sharding: tensor-parallel over heads (2 heads/core, both batch elements on
every core) for LN1/QKV/attention; one 8-core AllToAll redistributes the
per-head attention context to token-parallel shards (512 tokens/core) for the
output projection, LN2 and the FFN.  Matmuls run in bf16 with f32 PSUM
accumulation; the residual stream stays f32.

All layouts are transposed ([channel, token]) on chip so every matmul
contracts over the partition dim.  LayerNorm 1 is folded into the QKV weights:
q = inv_std[t] * (x @ Wq_eff - mu[t] * colsum(Wq_eff)) + be1 @ Wq, implemented
with a rank-2 correction matmul appended to each accumulation group.
"""

import numpy as np
import ml_dtypes

B, T, C = 2, 2048, 1024
NH, H = 16, 64
FF = 3 * C
EPS = 1e-6
N_CORES = 8
TT = B * T            # 4096 tokens processed per core (head-parallel phase)
TS = TT // N_CORES    # 512 tokens per core (token-parallel phase)
HPC = NH // N_CORES   # 2 heads per core
HD2 = HPC * H         # 128

BF16 = ml_dtypes.bfloat16

_BUILT = {}

NT = TT // 128        # 32 token tiles
NKC = C // 128        # 8 channel k-tiles
NMF = FF // 128       # 24 ff tiles


def _build():
    import concourse.bacc as bacc
    import concourse.mybir as mybir
    import concourse.tile as tile
    dt = mybir.dt
    alu = mybir.AluOpType
    act = mybir.ActivationFunctionType

    nc = bacc.Bacc("TRN2", target_bir_lowering=False, debug=False,
                   num_devices=N_CORES)

    # ----- kernel I/O (per-core shards) -----
    p_x = nc.declare_dram_parameter("p_x", [TT, C], dt.bfloat16, isOutput=False)
    p_xT = nc.declare_dram_parameter("p_xT", [C, TT], dt.bfloat16, isOutput=False)
    p_xs = nc.declare_dram_parameter("p_xs", [TS, C], dt.float32, isOutput=False)
    p_wq = nc.declare_dram_parameter("p_wq", [C, HD2], dt.bfloat16, isOutput=False)
    p_wk = nc.declare_dram_parameter("p_wk", [C, HD2], dt.bfloat16, isOutput=False)
    p_wv = nc.declare_dram_parameter("p_wv", [C, HD2], dt.bfloat16, isOutput=False)
    p_cq = nc.declare_dram_parameter("p_cq", [2, HD2], dt.bfloat16, isOutput=False)
    p_ck = nc.declare_dram_parameter("p_ck", [2, HD2], dt.bfloat16, isOutput=False)
    p_cv = nc.declare_dram_parameter("p_cv", [2, HD2], dt.bfloat16, isOutput=False)
    p_womov = nc.declare_dram_parameter("p_womov", [128, NKC * C], dt.bfloat16, isOutput=False)
    p_w1blk = nc.declare_dram_parameter("p_w1blk", [NMF, C, 128], dt.bfloat16, isOutput=False)
    p_b1c = nc.declare_dram_parameter("p_b1c", [128, NMF], dt.float32, isOutput=False)
    p_w2mov = nc.declare_dram_parameter("p_w2mov", [128, NMF * C], dt.bfloat16, isOutput=False)
    p_b2 = nc.declare_dram_parameter("p_b2", [1, C], dt.bfloat16, isOutput=False)
    p_maskw = nc.declare_dram_parameter("p_maskw", [128, 4 * 512], dt.bfloat16, isOutput=False)
    p_ind2 = nc.declare_dram_parameter("p_ind2", [2, 128], dt.bfloat16, isOutput=False)
    p_ident = nc.declare_dram_parameter("p_ident", [128, 128], dt.bfloat16, isOutput=False)
    p_out = nc.declare_dram_parameter("p_out", [TS, C], dt.float32, isOutput=True)

    with tile.TileContext(nc, num_cores=N_CORES) as tc:
        with (
            tc.tile_pool(name="persist", bufs=1) as pp,
            tc.tile_pool(name="dram", bufs=1, space="DRAM") as pdram,
        ):
            # Consumer-less first collective: absorbs the one-time
            # rendezvous (core-launch skew + comm init) while stage A runs.
            warm_in = pdram.tile([1, 16], dt.bfloat16)
            warm_out = pdram.tile([N_CORES, 1, 16], dt.bfloat16)
            nc.gpsimd.collective_compute(
                "AllGather", alu.bypass,
                replica_groups=[list(range(N_CORES))],
                ins=[warm_in.opt()],
                outs=[warm_out.opt()],
            )

            # ------------- persistent constants & activation tensors -------------
            # (tiles declared here; their DMAs are issued after the x^T/weight
            # DMAs so the first QKV matmul isn't stuck behind ~15 tiny loads)
            ident = pp.tile([128, 128], dt.bfloat16)
            maskw = pp.tile([128, 4, 512], dt.bfloat16)
            ones128_row = pp.tile([1, 128], dt.bfloat16)
            nc.vector.memset(ones128_row[:], 1.0)
            cq = pp.tile([2, HD2], dt.bfloat16)
            ck = pp.tile([2, HD2], dt.bfloat16)
            cv = pp.tile([2, HD2], dt.bfloat16)


            qT = pp.tile([128, TT], dt.bfloat16)
            kT = pp.tile([128, TT], dt.bfloat16)
            v = pp.tile([128, NT, 2, 65], dt.bfloat16)
            ctxT = pp.tile([128, TT], dt.bfloat16)

            # ---------------- stage A: LN1 stats (sharded) + QKV ----------------
            with (
                tc.tile_pool(name="xtpool", bufs=1) as pxt,
                tc.tile_pool(name="xin", bufs=4) as px,
                tc.tile_pool(name="stat", bufs=1) as pst,
                tc.tile_pool(name="apsum", bufs=3, space="PSUM") as pps_a,
                tc.tile_pool(name="apsum1", bufs=1, space="PSUM") as pps_a1,
            ):
                # x^T chunks 0-1 + QKV weights lead the DMA queue so the
                # first matmuls can start ~3us in.
                xT = pxt.tile([128, NKC, TT], dt.bfloat16)
                # first chunks split in half so two DMA queues fill them in
                # parallel and the first matmul starts sooner
                for ch in range(4):
                    for kh in range(2):
                        nc.sync.dma_start(
                            xT[:, 4 * kh:4 * (kh + 1), 512 * ch:512 * (ch + 1)],
                            p_xT.ap()[512 * kh:512 * (kh + 1),
                                      512 * ch:512 * (ch + 1)].rearrange(
                                "(k p) t -> p k t", p=128))
                wq = pst.tile([128, NKC, HD2], dt.bfloat16)
                nc.sync.dma_start(wq[:], p_wq.ap().rearrange("(k p) h -> p k h", p=128))
                wk = pst.tile([128, NKC, HD2], dt.bfloat16)
                nc.sync.dma_start(wk[:], p_wk.ap().rearrange("(k p) h -> p k h", p=128))
                wv = pst.tile([128, NKC, HD2], dt.bfloat16)
                nc.sync.dma_start(wv[:], p_wv.ap().rearrange("(k p) h -> p k h", p=128))
                nc.sync.dma_start(ident[:], p_ident[:])
                nc.sync.dma_start(maskw[:], p_maskw.ap().rearrange(
                    "p (o t) -> p o t", o=4))
                nc.sync.dma_start(cq[:], p_cq[:])
                nc.sync.dma_start(ck[:], p_ck[:])
                nc.sync.dma_start(cv[:], p_cv[:])

                # rows_all [2, TT]: row 0 = -mu, row 1 = std+eps
                rows_all = pst.tile([2, TT], dt.bfloat16)
                inv_row = pst.tile([1, TT], dt.bfloat16)
                inv_b = pst.tile([128, TT], dt.bfloat16)
                # LN1 stats for ALL tokens, computed redundantly per core
                # (no collective; the AllToAll is the only sync point)
                for sg in range(8):
                    # interleave the remaining x^T chunks with the stats x
                    # tiles so both DMA streams progress together
                    ch = sg + 4
                    if ch < TT // 512:
                        nc.sync.dma_start(
                            xT[:, :, 512 * ch:512 * (ch + 1)],
                            p_xT.ap()[:, 512 * ch:512 * (ch + 1)].rearrange(
                                "(k p) t -> p k t", p=128))
                    stats = px.tile([128, 4, 2], dt.float32, tag="stats")
                    for i in range(4):
                        ti = 4 * sg + i
                        xt = px.tile([128, C], dt.bfloat16, tag="xtc", bufs=8)
                        nc.sync.dma_start(xt[:], p_x[128 * ti:128 * (ti + 1), :])
                        bnt = px.tile([128, 2, 6], dt.float32, tag="bnt")
                        nc.vector.bn_stats(bnt[:, 0, :], xt[:, 0:512])
                        nc.vector.bn_stats(bnt[:, 1, :], xt[:, 512:1024])
                        nc.vector.bn_aggr(stats[:, i, :], bnt[:])
                    stat3 = px.tile([128, 4, 3], dt.bfloat16, tag="stat3")
                    stdf = px.tile([128, 4], dt.float32, tag="stdf")
                    nc.scalar.activation(stdf[:], stats[:, :, 1], act.Sqrt,
                                         scale=float(C) / (C - 1))
                    nc.vector.tensor_scalar(stdf[:], stdf[:], EPS, None, alu.add)
                    invf2 = px.tile([128, 4], dt.float32, tag="invf")
                    nc.vector.reciprocal(invf2[:], stdf[:])
                    nc.vector.tensor_scalar(stat3[:, :, 0], stats[:, :, 0],
                                            -1.0, None, alu.mult)
                    nc.vector.tensor_copy(stat3[:, :, 1], stdf[:])
                    nc.vector.tensor_copy(stat3[:, :, 2], invf2[:])
                    for i in range(4):
                        col = 512 * sg + 128 * i
                        pt = pps_a1.tile([2, 128], dt.bfloat16, tag="rowtp")
                        nc.tensor.transpose(pt[:], stat3[:, i, 0:2], ident[:])
                        nc.scalar.copy(rows_all[:, col:col + 128], pt[:])
                        ptv = pps_a1.tile([1, 128], dt.bfloat16, tag="rowtp")
                        nc.tensor.transpose(ptv[:], stat3[:, i, 2:3], ident[:])
                        nc.scalar.copy(inv_row[:, col:col + 128], ptv[:])

                # main QKV matmuls, independent of the LN1 stats exchange:
                # raw results parked in bf16, corrected once stats arrive.
                vT = pxt.tile([128, TT], dt.bfloat16)
                for cp in range(TT // 1024):
                    sls = [slice(1024 * cp, 1024 * cp + 512),
                           slice(1024 * cp + 512, 1024 * (cp + 1))]
                    for (nm, w, cw, dst) in (("q", wq, cq, qT), ("k", wk, ck, kT),
                                             ("v", wv, cv, vT)):
                        pss = [pps_a.tile([128, 512], dt.float32,
                                          name=f"ps{nm}{i}", tag="qkv", bufs=4)
                               for i in range(2)]
                        for k in range(NKC):
                            for i in range(2):
                                nc.tensor.matmul(pss[i][:], w[:, k, :],
                                                 xT[:, k, sls[i]],
                                                 start=(k == 0),
                                                 stop=(k == NKC - 1))
                        for i in range(2):
                            nc.scalar.copy(dst[:, sls[i]], pss[i][:])

                for ch in range(TT // 512):
                    pb = pps_a1.tile([128, 512], dt.float32, tag="rowtp")
                    nc.tensor.matmul(pb[:], ones128_row[:],
                                     inv_row[0:1, 512 * ch:512 * (ch + 1)],
                                     start=True, stop=True)
                    nc.scalar.copy(inv_b[:, 512 * ch:512 * (ch + 1)], pb[:])

                # rank-2 correction + 1/std scaling
                for ch in range(TT // 512):
                    sl = slice(512 * ch, 512 * (ch + 1))
                    for (nm, w, cw, dst) in (("q", wq, cq, qT), ("k", wk, ck, kT),
                                             ("v", wv, cv, vT)):
                        pc2 = pps_a.tile([128, 512], dt.float32,
                                         name=f"pc2{nm}", tag="corr", bufs=2)
                        nc.tensor.matmul(pc2[:], cw[:], rows_all[0:2, sl],
                                         start=True, stop=True)
                        t1 = px.tile([128, 512], dt.bfloat16, tag="t1", bufs=3)
                        nc.vector.tensor_tensor(t1[:], dst[:, sl], pc2[:],
                                                alu.add)
                        eng = nc.gpsimd if nm == "v" else nc.vector
                        eng.tensor_tensor(dst[:, sl], t1[:], inv_b[:, sl],
                                          alu.mult)

                # v_aug [s, tile, head, 65] via PE transpose of vT; col 64 = 1
                nc.vector.memset(v[:, :, :, 64], 1.0)
                for i in range(NT):
                    pvt = pps_a1.tile([128, 128], dt.bfloat16, tag="vtp")
                    nc.tensor.transpose(pvt[:], vT[:, 128 * i:128 * (i + 1)],
                                        ident[:])
                    nc.scalar.copy(v[:, i, :, 0:64],
                                   pvt[:].rearrange("p (h d) -> p h d", h=2))

            # -------- prefetch stage-C weights (overlaps attention) --------
            womov = pp.tile([128, NKC, C], dt.bfloat16)
            nc.sync.dma_start(womov[:],
                              p_womov.ap().rearrange("p (k c) -> p k c", k=NKC))
            xs = pp.tile([128, 4, C], dt.float32)
            nc.sync.dma_start(xs[:],
                              p_xs.ap().rearrange("(tt p) c -> p tt c", p=128))
            b2r = pp.tile([1, C], dt.bfloat16)
            nc.sync.dma_start(b2r[:], p_b2[:])
            b1c = pp.tile([128, NMF], dt.float32)
            nc.sync.dma_start(b1c[:], p_b1c[:])

            cc_in = pdram.tile([N_CORES, 128, TS], dt.bfloat16)
            cc_out = pdram.tile([N_CORES, 128, TS], dt.bfloat16)

            # ---------------- stage B: attention ----------------
            # indicator [2,128]: row h -> partitions 64h..64h+63
            ind2 = pp.tile([2, 128], dt.bfloat16)
            nc.sync.dma_start(ind2[:], p_ind2[:])
            # Scores for both heads share one [128,1024] PSUM tile (2 banks)
            # -> one exp per j.  Z-normalization is deferred: raw ctx + z rows
            # are evicted per group, one batched reciprocal per batch half,
            # normalize overlapped with the next batch's scores.
            with (
                tc.tile_pool(name="exps", bufs=8) as pexp,
                tc.tile_pool(name="attsb", bufs=2) as pat,
                tc.tile_pool(name="battn", bufs=1) as pbat,
                tc.tile_pool(name="scpsum", bufs=3, space="PSUM") as pps_sc,
                tc.tile_pool(name="ctxpsum", bufs=1, space="PSUM") as pps_ctx,
            ):
                ctxu = pbat.tile([128, TT], dt.bfloat16)  # unnormalized ctx
                # z rows: batch b at partitions 32b..32b+7 (32-aligned reads)
                ZROW = {g: 32 * (g // 4) + 2 * (g % 4) for g in range(8)}
                zall = pbat.tile([40, 512], dt.float32)
                zinvf = pbat.tile([40, 512], dt.float32)
                zinv16 = pbat.tile([40, 512], dt.bfloat16)

                def group_epilogue(b):
                    # 1/z for this batch's 8 (group, head) rows, then scale.
                    # Batch 1 runs after all scores -> may reuse the sc psum;
                    # batch 0 must not disturb the live score pipeline.
                    r0 = 32 * b
                    tag, pool = "sc", pps_sc
                    nc.vector.reciprocal(zinvf[r0:r0 + 8, :],
                                         zall[r0:r0 + 8, :])
                    nc.vector.tensor_copy(zinv16[r0:r0 + 8, :],
                                          zinvf[r0:r0 + 8, :])
                    zgbs = []
                    for qt in range(4):
                        zgb = pat.tile([2, 512], dt.bfloat16, tag="zgb",
                                       bufs=4)
                        nc.sync.dma_start(
                            zgb[:], zinv16[r0 + 2 * qt:r0 + 2 * qt + 2, :])
                        zgbs.append(zgb)
                    for qt in range(4):
                        g = b * 4 + qt
                        gsl = slice(512 * g, 512 * (g + 1))
                        pzb = pool.tile([128, 512], dt.float32, tag=tag,
                                        name="pzb")
                        nc.tensor.matmul(pzb[:], ind2[:], zgbs[qt][:],
                                         start=True, stop=True)
                        zb = pat.tile([128, 512], dt.bfloat16, tag="zbs")
                        nc.vector.tensor_copy(zb[:], pzb[:])
                        nc.vector.tensor_tensor(ctxT[:, gsl], ctxu[:, gsl],
                                                zb[:], alu.mult)
                        nc.sync.dma_start(cc_in[g], ctxT[:, gsl])

                for b in range(B):
                    for qt in range(T // 512):
                        G = b * T + 512 * qt
                        g = b * 4 + qt
                        gsl = slice(G, G + 512)
                        nj = 4 * qt + 4
                        pc = [pps_ctx.tile([65, 512], dt.float32,
                                           name=f"pc{h}", tag=f"ctx{h}")
                              for h in range(2)]
                        ets = []
                        for j in range(nj):
                            st = b * (T // 128) + j   # global s-tile index
                            ps = pps_sc.tile([128, 1024], dt.float32, tag="sc",
                                             name="ps")
                            for h in range(2):
                                hsl = slice(64 * h, 64 * (h + 1))
                                nc.tensor.matmul(
                                    ps[:, 512 * h:512 * (h + 1)],
                                    kT[hsl, 128 * st:128 * (st + 1)],
                                    qT[hsl, gsl], start=True, stop=True)
                            et = pexp.tile([128, 1024], dt.bfloat16, tag="et",
                                           name="et")
                            if j >= nj - 4:
                                off = j - (nj - 4)
                                if off > 0:
                                    nc.gpsimd.memset(
                                        et[:].rearrange("p (h t) -> p h t",
                                                        h=2)[:, :, 0:128 * off],
                                        0.0)
                                for h in range(2):
                                    o = 512 * h + 128 * off
                                    nc.scalar.activation(
                                        et[:, o:512 * (h + 1)],
                                        ps[:, o:512 * (h + 1)],
                                        act.Exp, scale=1.0 / float(np.sqrt(H)))
                                    nc.vector.tensor_tensor(
                                        et[:, o:o + 128], et[:, o:o + 128],
                                        maskw[:, off, 128 * off:128 * (off + 1)],
                                        alu.mult)
                            else:
                                nc.scalar.activation(et[:], ps[:], act.Exp,
                                                     scale=1.0 / float(np.sqrt(H)))
                            ets.append(et)
                            # software pipeline: AV for tile j-2 after scores of j
                            if j >= 2:
                                for h in range(2):
                                    nc.tensor.matmul(
                                        pc[h][:], v[:, b * (T // 128) + j - 2, h, :],
                                        ets[j - 2][:, 512 * h:512 * (h + 1)],
                                        start=(j - 2 == 0), stop=False)
                        for jt in (nj - 2, nj - 1):
                            for h in range(2):
                                nc.tensor.matmul(
                                    pc[h][:], v[:, b * (T // 128) + jt, h, :],
                                    ets[jt][:, 512 * h:512 * (h + 1)],
                                    start=(jt == 0), stop=(jt == nj - 1))
                        # evict raw ctx + z row; normalization is deferred
                        for h in range(2):
                            zrow = pat.tile([1, 512], dt.float32, tag="zr",
                                            bufs=4)
                            nc.vector.tensor_copy(zrow[:], pc[h][64:65, :])
                            r = ZROW[g] + h
                            nc.sync.dma_start(zall[r:r + 1, :], zrow[:])
                            nc.vector.tensor_copy(
                                ctxu[64 * h:64 * (h + 1), gsl], pc[h][0:64, :])
                        if g == 4:
                            group_epilogue(0)
                if True:
                    group_epilogue(1)

            # ---------------- AllToAll: heads -> tokens ----------------
            nc.gpsimd.collective_compute(
                "AllToAll", alu.bypass,
                replica_groups=[list(range(N_CORES))],
                ins=[cc_in.opt()],
                outs=[cc_out.opt()],
            )

            # ------- stage C: Wo + LN2 + FFN, [token, channel] layout -------
            with (
                tc.tile_pool(name="postsb", bufs=1) as pq,
                tc.tile_pool(name="wstream", bufs=2) as pw,
                tc.tile_pool(name="evict", bufs=3) as pev,
                tc.tile_pool(name="ln2", bufs=1) as pl2,
            ):
                w2mov = pq.tile([128, NMF, C], dt.bfloat16)
                nc.sync.dma_start(w2mov[:],
                                  p_w2mov.ap().rearrange("p (k c) -> p k c",
                                                         k=NMF))
                ctxF = pq.tile([128, NKC, TS], dt.bfloat16)
                for j in range(N_CORES):
                    nc.sync.dma_start(ctxF[:, j, :], cc_out[j])

                r2 = pq.tile([128, 4, C], dt.float32)     # x + bo + attn, [tok, c]
                xn2T = pq.tile([128, NKC, TS], dt.bfloat16)
                st2 = pl2.tile([128, 4, 2], dt.float32)
                stdt = pl2.tile([128, 4], dt.float32)
                inv2c = pl2.tile([128, 4], dt.float32)
                negmu = pl2.tile([128, 4], dt.float32)

                with (
                    tc.tile_pool(name="wops", bufs=4, space="PSUM") as pps_wo,
                    tc.tile_pool(name="tpps", bufs=4, space="PSUM") as pps_tp,
                ):
                    # Wo: stationary ctx blocks, moving WoT; out [tok, c]
                    for tt in range(4):
                        tsl = slice(128 * tt, 128 * (tt + 1))
                        pwos = [pps_wo.tile([128, 512], dt.float32, tag="wo",
                                            name=f"pwo{cb}")
                                for cb in range(2)]
                        for k in range(NKC):
                            for cb in range(2):
                                csl = slice(512 * cb, 512 * (cb + 1))
                                nc.tensor.matmul(pwos[cb][:], ctxF[:, k, tsl],
                                                 womov[:, k, csl],
                                                 start=(k == 0),
                                                 stop=(k == NKC - 1))
                        for cb in range(2):
                            csl = slice(512 * cb, 512 * (cb + 1))
                            nc.vector.tensor_tensor(r2[:, tt, csl], pwos[cb][:],
                                                    xs[:, tt, csl], alu.add)
                        # LN2 stats for this token tile
                        bnt = pl2.tile([128, 2, 6], dt.float32, tag="bnt",
                                       bufs=2)
                        nc.vector.bn_stats(bnt[:, 0, :], r2[:, tt, 0:512])
                        nc.vector.bn_stats(bnt[:, 1, :], r2[:, tt, 512:1024])
                        nc.vector.bn_aggr(st2[:, tt, :], bnt[:])

                    # (x - mu) / (std + eps), torch ddof=1
                    nc.scalar.activation(stdt[:], st2[:, :, 1], act.Sqrt,
                                         scale=float(C) / (C - 1))
                    nc.vector.tensor_scalar(stdt[:], stdt[:], EPS, None, alu.add)
                    nc.vector.reciprocal(inv2c[:], stdt[:])
                    nc.vector.tensor_scalar(negmu[:], st2[:, :, 0], -1.0, None,
                                            alu.mult)
                    for tt in range(4):
                        xn2 = pev.tile([128, C], dt.bfloat16, tag="xn2",
                                       bufs=2)
                        nc.vector.tensor_scalar(xn2[:], r2[:, tt, :],
                                                negmu[:, tt:tt + 1],
                                                inv2c[:, tt:tt + 1],
                                                alu.add, alu.mult)
                        # transpose to [c, tok] for FFN1
                        for kc in range(NKC):
                            pt = pps_tp.tile([128, 128], dt.bfloat16, tag="tp")
                            nc.tensor.transpose(
                                pt[:], xn2[:, 128 * kc:128 * (kc + 1)],
                                ident[:])
                            nc.scalar.copy(xn2T[:, kc, 128 * tt:128 * (tt + 1)],
                                           pt[:])

                # FFN1: stationary W1 blocks -> hT [ff, tok]
                hT = pq.tile([128, NMF, TS], dt.bfloat16)
                with tc.tile_pool(name="ffps", bufs=4, space="PSUM") as pps_ff:
                    for mf in range(NMF):
                        w1_blk = pw.tile([128, NKC, 128], dt.bfloat16, tag="w1",
                                         bufs=8)
                        nc.sync.dma_start(
                            w1_blk[:],
                            p_w1blk[mf].rearrange("(k p) f -> p k f", p=128))
                        pf = pps_ff.tile([128, TS], dt.float32, tag="ff")
                        for k in range(NKC):
                            nc.tensor.matmul(pf[:], w1_blk[:, k, :],
                                             xn2T[:, k, :],
                                             start=(k == 0),
                                             stop=(k == NKC - 1))
                        nc.scalar.activation(hT[:, mf, :], pf[:], act.Relu,
                                             bias=b1c[:, mf:mf + 1])

                # FFN2: stationary hT blocks, moving W2T; out [tok, c]
                with tc.tile_pool(name="ff2ps", bufs=8, space="PSUM") as pps_f2:
                    for tt in range(4):
                        tsl = slice(128 * tt, 128 * (tt + 1))
                        pos = [pps_f2.tile([128, 512], dt.float32, tag="f2",
                                           name=f"po{cb}") for cb in range(2)]
                        for k in range(NMF):
                            for cb in range(2):
                                csl = slice(512 * cb, 512 * (cb + 1))
                                nc.tensor.matmul(pos[cb][:], hT[:, k, tsl],
                                                 w2mov[:, k, csl],
                                                 start=(k == 0), stop=False)
                        for cb in range(2):
                            csl = slice(512 * cb, 512 * (cb + 1))
                            nc.tensor.matmul(pos[cb][:], ones128_row[:],
                                             b2r[0:1, csl],
                                             start=False, stop=True)
                            ot = pev.tile([128, 512], dt.float32, tag="ot")
                            nc.vector.tensor_tensor(ot[:], pos[cb][:],
                                                    r2[:, tt, csl], alu.add)
                            nc.sync.dma_start(p_out[tsl, csl], ot[:])

    nc.compile()
    return nc


def _host_prep(inputs):
    """Fold layernorm affine params into weights; build per-core input maps."""
    x = np.asarray(inputs["x"], np.float32)
    Wq = np.asarray(inputs["Wq"], np.float32)
    Wk = np.asarray(inputs["Wk"], np.float32)
    Wv = np.asarray(inputs["Wv"], np.float32)
    Wo = np.asarray(inputs["Wo"], np.float32)
    bo = np.asarray(inputs["bo"], np.float32)
    W1 = np.asarray(inputs["W1"], np.float32)
    b1 = np.asarray(inputs["b1"], np.float32)
    W2 = np.asarray(inputs["W2"], np.float32)
    b2 = np.asarray(inputs["b2"], np.float32)
    g1 = np.asarray(inputs["g1"], np.float32)
    be1 = np.asarray(inputs["be1"], np.float32)
    g2 = np.asarray(inputs["g2"], np.float32)
    be2 = np.asarray(inputs["be2"], np.float32)

    xf = x.reshape(TT, C)                      # both batches stacked
    xT = np.ascontiguousarray(xf.T)            # [C, TT]

    def fold_qkv(W):
        Weff = g1[:, None] * W                  # [NH, C, H] with g1 on C
        Weff = np.ascontiguousarray(np.transpose(Weff, (1, 0, 2)))  # [C, NH, H]
        bias = np.einsum("c,hck->hk", be1, W)   # [NH, H]
        colsum = Weff.sum(axis=0)               # [NH, H]
        return Weff, bias, colsum

    Wq_e, bq, csq = fold_qkv(Wq)
    Wk_e, bk, csk = fold_qkv(Wk)
    Wv_e, bv, csv = fold_qkv(Wv)

    woT = np.ascontiguousarray(Wo.T)            # [NH*H, C]
    w1T = np.ascontiguousarray(g2[:, None] * W1.T)   # [C, FF]
    b1_eff = b1 + be2 @ W1.T                         # [FF]
    w2T = np.ascontiguousarray(W2.T)            # [FF, C]

    w1blk = np.ascontiguousarray(
        w1T.reshape(C, NMF, 128).transpose(1, 0, 2))
    womov = np.ascontiguousarray(
        woT.reshape(NKC, 128, C).transpose(1, 0, 2)).reshape(128, NKC * C)
    w2mov = np.ascontiguousarray(
        w2T.reshape(NMF, 128, C).transpose(1, 0, 2)).reshape(128, NMF * C)

    tq = np.arange(128)[None, :]
    s = np.arange(128)[:, None]
    maskd = (s <= tq).astype(np.float32)
    maskw = np.zeros((128, 4, 512), np.float32)
    for off in range(4):
        maskw[:, off, 128 * off:128 * (off + 1)] = maskd

    x_bf = xf.astype(BF16)
    shared = {
        "p_x": x_bf,
        "p_xT": xT.astype(BF16),
        "p_womov": womov.astype(BF16),
        "p_w1blk": w1blk.astype(BF16),
        "p_b1c": np.ascontiguousarray(
            b1_eff.reshape(NMF, 128).T).astype(np.float32),
        "p_w2mov": w2mov.astype(BF16),
        "p_b2": b2[None, :].astype(BF16),
        "p_maskw": maskw.reshape(128, 4 * 512).astype(BF16),
        "p_ind2": np.repeat(np.eye(2, dtype=np.float32), 64, axis=1).astype(BF16),
        "p_ident": np.eye(128, dtype=np.float32).astype(BF16),
    }

    in_maps = []
    for r in range(N_CORES):
        h0 = HPC * r
        hs = slice(h0, h0 + HPC)
        b_r, s_r = divmod(r, N_CORES // B)
        tok = slice(s_r * TS, (s_r + 1) * TS)
        m = dict(shared)
        m["p_xs"] = (x[b_r][tok, :] + bo[None, :]).astype(np.float32)
        m["p_wq"] = np.ascontiguousarray(
            Wq_e[:, hs, :].reshape(C, HD2)).astype(BF16)
        m["p_wk"] = np.ascontiguousarray(
            Wk_e[:, hs, :].reshape(C, HD2)).astype(BF16)
        m["p_wv"] = np.ascontiguousarray(
            Wv_e[:, hs, :].reshape(C, HD2)).astype(BF16)
        m["p_cq"] = np.stack([csq[hs].reshape(HD2),
                              bq[hs].reshape(HD2)]).astype(BF16)
        m["p_ck"] = np.stack([csk[hs].reshape(HD2),
                              bk[hs].reshape(HD2)]).astype(BF16)
        m["p_cv"] = np.stack([csv[hs].reshape(HD2),
                              bv[hs].reshape(HD2)]).astype(BF16)
        in_maps.append(m)
    return in_maps


def kernel(**inputs) -> np.ndarray:
    from concourse.bass_utils import run_bass_kernel_spmd

    if "nc" not in _BUILT:
        _BUILT["nc"] = _build()
    nc = _BUILT["nc"]

    in_maps = _host_prep(inputs)
    res = run_bass_kernel_spmd(nc, in_maps, core_ids=list(range(N_CORES)))

    out = np.empty((B, T, C), np.float32)
    for r in range(N_CORES):
        b_r, s_r = divmod(r, N_CORES // B)
        out[b_r, s_r * TS:(s_r + 1) * TS, :] = res.results[r]["p_out"]
    return out

